# revision 18
# baseline (speedup 1.0000x reference)
"""Trainium2 Bass kernel for nn_LsqNonneg: batched NNLS via heavy-ball projected gradient.

Math: the reference runs 200 FISTA iterations converging to the NNLS solution
S* (within ~3e-3 of it).  We converge to the same fixed point with a warm
start + constant-momentum heavy-ball iteration, which needs only ~29 steps:

    AtA = A.T A,  eigs: L = lam_max, mu = lam_min
    alpha = 4/(sqrt(L)+sqrt(mu))^2,  beta = ((sqrt(k)-1)/(sqrt(k)+1))^2, k=L/mu
    B   = alpha * A.T X                    [32, N]
    S0  = relu((1/L) A.T X) = relu(c0*B),  c0 = 1/(L*alpha)
    S1  = relu(W S0 + B),                  W  = I - alpha*AtA
    S_{k+1} = relu(Wc S_k - beta*S_{k-1} + B),  Wc = (1+beta)I - alpha*AtA

All iteration weights are constant -> loaded once, no per-iteration streaming.

Device layout (per core, NS=4096 columns): packed [128, 512] per slice s:
partition group g (rows 32g..32g+31) of slice s holds original columns
[g*1024 + 512*s, g*1024 + 512*s + 512).  Weights are diag4 [128,128] blocks so
one full-array matmul advances 4 column blocks; one slice = one PSUM bank.
Per step per slice: 3 accumulating matmuls in order (ident@B start, wp@S_{k-1},
wc@S_k stop -- the relu-gated matmul goes last so the two ungated ones absorb
the relu latency), then relu psum->S (slice 0 on VectorE, slice 1 on ScalarE)
overlapped with the other slice's matmuls.  X is staged in fp16 (host-cast) to
halve the DMA-bound prologue; B accumulates in fp32 PSUM so the data term
keeps full precision.
"""

import os
import sys

import numpy as np

for _p in ("/opt/trn_rl_repo", "/root/.axon_site/_ro/trn_rl_repo"):
    if os.path.isdir(_p) and _p not in sys.path:
        sys.path.append(_p)

from contextlib import ExitStack

import concourse.bass as bass
import concourse.bacc as bacc
import concourse.tile as tile
from concourse import mybir
from concourse.bass_utils import run_bass_kernel_spmd

M, K, N_FULL, N_CORES = 512, 32, 32768, 8
NSTEP = 29               # heavy-ball steps after the warm start

F32 = mybir.dt.float32
F32R = mybir.dt.float32r
F16 = mybir.dt.float16

MM_DTYPE = F32R

LAST_RESULTS = None  # BassKernelResults of the most recent run (for test.py)


def build_program(ns: int, nstep: int, c0: float, alpha: float, mm_dtype=MM_DTYPE):
    """Build the SPMD Bass program for one core holding `ns` columns."""
    DT = mm_dtype
    assert ns == 4096
    SL = 512             # columns per slice (one PSUM bank)
    NSL = 2              # slices

    nc = bacc.Bacc("TRN2", target_bir_lowering=False)

    x_d = nc.dram_tensor("x", [M, ns], F16, kind="ExternalInput")
    apad_d = nc.dram_tensor("apad", [128, 16 * 128], F16, kind="ExternalInput")
    wts_d = nc.dram_tensor("wts", [128, 4 * 128], F32, kind="ExternalInput")
    out_d = nc.dram_tensor("s_out", [K, ns], F32, kind="ExternalOutput")

    with ExitStack() as ctx:
        tc = ctx.enter_context(tile.TileContext(nc))
        persist = ctx.enter_context(tc.tile_pool(name="persist", bufs=1))
        psum = ctx.enter_context(tc.tile_pool(name="psum", bufs=4, space="PSUM"))

        # --- X staging first (critical path), natural layout, fp16 ---
        xts = [persist.tile([128, ns], F16, name=f"xt{c}") for c in range(4)]
        for c in range(4):
            nc.sync.dma_start(xts[c][:], x_d[128 * c:128 * (c + 1), :])

        # --- constants on the scalar HWDGE ring (parallel with X) ---
        apc = persist.tile([128, 16 * 128], F16)  # (g,c) chunk at 128*(4g+c)
        nc.scalar.dma_start(apc[:], apad_d[:])
        wts_sb = persist.tile([128, 4 * 128], DT)
        nc.scalar.dma_start(wts_sb[:], wts_d[:].bitcast(DT))
        id_sb = wts_sb[:, 0:128]
        w1_sb = wts_sb[:, 128:256]
        wc_sb = wts_sb[:, 256:384]
        wp_sb = wts_sb[:, 384:512]

        # --- B = alpha * A.T X  (packed layout), per slice ---
        pb = [psum.tile([128, SL], F32, tag=f"pt{s}", name=f"pb{s}", bufs=4) for s in range(NSL)]
        for c in range(4):
            for s in range(NSL):
                for g in range(4):
                    nc.tensor.matmul(
                        pb[s][:],
                        apc[:, 128 * (4 * g + c):128 * (4 * g + c + 1)],
                        xts[c][:, 1024 * g + SL * s: 1024 * g + SL * (s + 1)],
                        start=(c == 0 and g == 0),
                        stop=(c == 3 and g == 3),
                    )

        b_sb = [persist.tile([128, SL], DT, name=f"b_sb{s}") for s in range(NSL)]
        s_a = [persist.tile([128, SL], DT, name=f"s_a{s}") for s in range(NSL)]
        s_b = [persist.tile([128, SL], DT, name=f"s_b{s}") for s in range(NSL)]

        # drain B = alpha * (A.T X psum) to SBUF, warm-start S0 = relu(c0 * B)
        nc.vector.tensor_scalar_mul(b_sb[0][:], pb[0][:], alpha)
        nc.scalar.activation(b_sb[1][:], pb[1][:],
                             mybir.ActivationFunctionType.Copy, scale=alpha)
        nc.vector.tensor_scalar(s_a[0][:], b_sb[0][:], c0, 0.0,
                                mybir.AluOpType.mult, mybir.AluOpType.max)
        nc.vector.tensor_scalar(s_a[1][:], b_sb[1][:], c0, 0.0,
                                mybir.AluOpType.mult, mybir.AluOpType.max)

        # --- heavy-ball loop: step k computes S_{k+1} (S_k in cur) ---
        for k in range(1, nstep + 1):
            cur = s_a if k % 2 == 1 else s_b
            dest = s_b if k % 2 == 1 else s_a
            pts = []
            for s in range(NSL):
                pt = psum.tile([128, SL], F32, tag=f"pt{s}", name=f"pt{s}", bufs=4)
                nc.tensor.matmul(pt[:], id_sb, b_sb[s][:],
                                 start=True, stop=False)
                if k == 1:
                    nc.tensor.matmul(pt[:], w1_sb, cur[s][:],
                                     start=False, stop=True)
                else:
                    nc.tensor.matmul(pt[:], wp_sb, dest[s][:],
                                     start=False, stop=False)
                    nc.tensor.matmul(pt[:], wc_sb, cur[s][:],
                                     start=False, stop=True)
                pts.append(pt)
            nc.vector.tensor_scalar_max(dest[0][:], pts[0][:], 0.0)
            nc.scalar.activation(dest[1][:], pts[1][:],
                                 mybir.ActivationFunctionType.Relu)

        final = s_a if nstep % 2 == 0 else s_b
        for s in range(NSL):
            for g in range(4):
                eng = nc.sync if g % 2 == 0 else nc.scalar
                eng.dma_start(
                    out_d[:, g * 1024 + SL * s: g * 1024 + SL * (s + 1)],
                    final[s][K * g:K * (g + 1), :].bitcast(F32),
                )

    nc.finalize()
    return nc


def host_prep(A: np.ndarray, nstep: int):
    """Heavy-ball coefficients + constant device weights from A."""
    A = np.asarray(A, dtype=np.float32)
    AtA = (A.T @ A).astype(np.float64)
    ev = np.linalg.eigvalsh(AtA)
    L, mu = float(ev[-1]), float(ev[0])
    kap = L / mu
    alpha = 4.0 / (np.sqrt(L) + np.sqrt(mu)) ** 2
    beta = ((np.sqrt(kap) - 1.0) / (np.sqrt(kap) + 1.0)) ** 2
    c0 = (1.0 / L) / alpha

    W1 = (np.eye(K) - alpha * AtA)
    Wc = ((1.0 + beta) * np.eye(K) - alpha * AtA)

    def diag4(Wt):
        out = np.zeros((128, 128), dtype=np.float32)
        for g in range(4):
            out[K * g:K * (g + 1), K * g:K * (g + 1)] = Wt.astype(np.float32)
        return out

    w1 = diag4(W1.T)
    wc = diag4(Wc.T)
    wp = (-beta * np.eye(128)).astype(np.float32)
    idm = np.eye(128, dtype=np.float32)

    A16 = A.astype(np.float16)
    apad = np.zeros((128, 16 * 128), dtype=np.float16)
    for g in range(4):
        for c in range(4):
            blk = np.zeros((128, 128), dtype=np.float16)
            blk[:, K * g:K * (g + 1)] = A16[128 * c:128 * (c + 1), :]
            apad[:, 128 * (4 * g + c):128 * (4 * g + c + 1)] = blk
    wts = np.concatenate([idm, w1, wc, wp], axis=1)
    return apad, wts, float(c0), float(alpha)


_PROGRAM_CACHE = {}


def _get_program(ns, nstep, c0, alpha):
    key = (ns, nstep, round(c0, 10), round(alpha, 12), str(MM_DTYPE))
    if key not in _PROGRAM_CACHE:
        _PROGRAM_CACHE[key] = build_program(ns, nstep, c0, alpha)
    return _PROGRAM_CACHE[key]


def kernel(X: np.ndarray, A: np.ndarray) -> np.ndarray:
    global LAST_RESULTS
    X = np.ascontiguousarray(np.asarray(X, dtype=np.float32))
    A = np.ascontiguousarray(np.asarray(A, dtype=np.float32))
    assert X.shape == (M, N_FULL) and A.shape == (M, K)

    ns = N_FULL // N_CORES
    apad, wts, c0, alpha = host_prep(A, NSTEP)
    nc = _get_program(ns, NSTEP, c0, alpha)

    in_maps = []
    for c in range(N_CORES):
        in_maps.append({
            "x": np.ascontiguousarray(X[:, c * ns:(c + 1) * ns].astype(np.float16)),
            "apad": apad,
            "wts": wts,
        })

    res = run_bass_kernel_spmd(nc, in_maps, core_ids=list(range(N_CORES)))
    LAST_RESULTS = res
    S = np.concatenate([res.results[c]["s_out"] for c in range(N_CORES)], axis=1)
    return np.ascontiguousarray(S.astype(np.float32))


# revision 19
# speedup vs baseline: 1.0221x; 1.0221x over previous
"""Trainium2 Bass kernel for nn_LsqNonneg: batched NNLS via heavy-ball projected gradient.

Math: the reference runs 200 FISTA iterations converging to the NNLS solution
S* (within ~3e-3 of it).  We converge to the same fixed point with a warm
start + constant-momentum heavy-ball iteration, which needs only ~29 steps:

    AtA = A.T A,  eigs: L = lam_max, mu = lam_min
    alpha = 4/(sqrt(L)+sqrt(mu))^2,  beta = ((sqrt(k)-1)/(sqrt(k)+1))^2, k=L/mu
    B   = alpha * A.T X                    [32, N]
    S0  = relu((1/L) A.T X) = relu(c0*B),  c0 = 1/(L*alpha)
    S1  = relu(W S0 + B),                  W  = I - alpha*AtA
    S_{k+1} = relu(Wc S_k - beta*S_{k-1} + B),  Wc = (1+beta)I - alpha*AtA

All iteration weights are constant -> loaded once, no per-iteration streaming.

Device layout (per core, NS=4096 columns): packed [128, 512] per slice s:
partition group g (rows 32g..32g+31) of slice s holds original columns
[g*1024 + 512*s, g*1024 + 512*s + 512).  Weights are diag4 [128,128] blocks so
one full-array matmul advances 4 column blocks; one slice = one PSUM bank.
Per step per slice: 3 accumulating matmuls in order (ident@B start, wp@S_{k-1},
wc@S_k stop -- the relu-gated matmul goes last so the two ungated ones absorb
the relu latency), then relu psum->S (slice 0 on VectorE, slice 1 on ScalarE)
overlapped with the other slice's matmuls.  X is staged in fp16 (host-cast) to
halve the DMA-bound prologue; B accumulates in fp32 PSUM so the data term
keeps full precision.
"""

import os
import sys

import numpy as np

for _p in ("/opt/trn_rl_repo", "/root/.axon_site/_ro/trn_rl_repo"):
    if os.path.isdir(_p) and _p not in sys.path:
        sys.path.append(_p)

from contextlib import ExitStack

import concourse.bass as bass
import concourse.bacc as bacc
import concourse.tile as tile
from concourse import mybir
from concourse.bass_utils import run_bass_kernel_spmd

M, K, N_FULL, N_CORES = 512, 32, 32768, 8
NSTEP = 29               # heavy-ball steps after the warm start

F32 = mybir.dt.float32
F32R = mybir.dt.float32r
F16 = mybir.dt.float16

MM_DTYPE = F32R

LAST_RESULTS = None  # BassKernelResults of the most recent run (for test.py)


def build_program(ns: int, nstep: int, c0: float, alpha: float, mm_dtype=MM_DTYPE):
    """Build the SPMD Bass program for one core holding `ns` columns."""
    DT = mm_dtype
    assert ns == 4096
    SL = 512             # columns per slice (one PSUM bank)
    NSL = 2              # slices

    nc = bacc.Bacc("TRN2", target_bir_lowering=False)

    x_d = nc.dram_tensor("x", [M, ns], F16, kind="ExternalInput")
    apad_d = nc.dram_tensor("apad", [128, 16 * 128], F16, kind="ExternalInput")
    wts_d = nc.dram_tensor("wts", [128, 4 * 128], F32, kind="ExternalInput")
    out_d = nc.dram_tensor("s_out", [K, ns], F32, kind="ExternalOutput")

    with ExitStack() as ctx:
        tc = ctx.enter_context(tile.TileContext(nc))
        persist = ctx.enter_context(tc.tile_pool(name="persist", bufs=1))
        psum = ctx.enter_context(tc.tile_pool(name="psum", bufs=4, space="PSUM"))

        # --- X staging first (critical path), natural layout, fp16 ---
        xts = [persist.tile([128, ns], F16, name=f"xt{c}") for c in range(4)]
        for c in range(4):
            nc.sync.dma_start(xts[c][:], x_d[128 * c:128 * (c + 1), :])

        # --- constants on the scalar HWDGE ring (parallel with X) ---
        apc = persist.tile([128, 16 * 128], F16)  # (g,c) chunk at 128*(4g+c)
        nc.scalar.dma_start(apc[:], apad_d[:])
        wts_sb = persist.tile([128, 4 * 128], DT)
        nc.scalar.dma_start(wts_sb[:], wts_d[:].bitcast(DT))
        id_sb = wts_sb[:, 0:128]
        w1_sb = wts_sb[:, 128:256]
        wc_sb = wts_sb[:, 256:384]
        wp_sb = wts_sb[:, 384:512]

        # --- B = alpha * A.T X  (packed layout), per slice ---
        pb = [psum.tile([128, SL], F32, tag=f"pt{s}", name=f"pb{s}", bufs=4) for s in range(NSL)]
        for c in range(4):
            for s in range(NSL):
                for g in range(4):
                    nc.tensor.matmul(
                        pb[s][:],
                        apc[:, 128 * (4 * g + c):128 * (4 * g + c + 1)],
                        xts[c][:, 1024 * g + SL * s: 1024 * g + SL * (s + 1)],
                        start=(c == 0 and g == 0),
                        stop=(c == 3 and g == 3),
                    )

        b_sb = [persist.tile([128, SL], DT, name=f"b_sb{s}") for s in range(NSL)]
        s_a = [persist.tile([128, SL], DT, name=f"s_a{s}") for s in range(NSL)]
        s_b = [persist.tile([128, SL], DT, name=f"s_b{s}") for s in range(NSL)]

        # drain B = alpha * (A.T X psum) to SBUF, warm-start S0 = relu(c0 * B)
        # slice 1's warm reads its PSUM directly on ScalarE so neither warm
        # blocks the VectorE relu stream on a cross-engine dependency.
        nc.vector.tensor_scalar_mul(b_sb[0][:], pb[0][:], alpha)
        nc.scalar.activation(b_sb[1][:], pb[1][:],
                             mybir.ActivationFunctionType.Copy, scale=alpha)
        nc.vector.tensor_scalar(s_a[0][:], b_sb[0][:], c0, 0.0,
                                mybir.AluOpType.mult, mybir.AluOpType.max)
        nc.scalar.activation(s_a[1][:], pb[1][:],
                             mybir.ActivationFunctionType.Relu,
                             scale=c0 * alpha)

        # --- heavy-ball loop: step k computes S_{k+1} (S_k in cur) ---
        for k in range(1, nstep + 1):
            cur = s_a if k % 2 == 1 else s_b
            dest = s_b if k % 2 == 1 else s_a
            pts = []
            for s in range(NSL):
                pt = psum.tile([128, SL], F32, tag=f"pt{s}", name=f"pt{s}", bufs=4)
                nc.tensor.matmul(pt[:], id_sb, b_sb[s][:],
                                 start=True, stop=False)
                if k == 1:
                    nc.tensor.matmul(pt[:], w1_sb, cur[s][:],
                                     start=False, stop=True)
                else:
                    nc.tensor.matmul(pt[:], wp_sb, dest[s][:],
                                     start=False, stop=False)
                    nc.tensor.matmul(pt[:], wc_sb, cur[s][:],
                                     start=False, stop=True)
                pts.append(pt)
            nc.vector.tensor_scalar_max(dest[0][:], pts[0][:], 0.0)
            nc.scalar.activation(dest[1][:], pts[1][:],
                                 mybir.ActivationFunctionType.Relu)

        final = s_a if nstep % 2 == 0 else s_b
        for s in range(NSL):
            for g in range(4):
                eng = nc.sync if g % 2 == 0 else nc.scalar
                eng.dma_start(
                    out_d[:, g * 1024 + SL * s: g * 1024 + SL * (s + 1)],
                    final[s][K * g:K * (g + 1), :].bitcast(F32),
                )

    nc.finalize()
    return nc


def host_prep(A: np.ndarray, nstep: int):
    """Heavy-ball coefficients + constant device weights from A."""
    A = np.asarray(A, dtype=np.float32)
    AtA = (A.T @ A).astype(np.float64)
    ev = np.linalg.eigvalsh(AtA)
    L, mu = float(ev[-1]), float(ev[0])
    kap = L / mu
    alpha = 4.0 / (np.sqrt(L) + np.sqrt(mu)) ** 2
    beta = ((np.sqrt(kap) - 1.0) / (np.sqrt(kap) + 1.0)) ** 2
    c0 = (1.0 / L) / alpha

    W1 = (np.eye(K) - alpha * AtA)
    Wc = ((1.0 + beta) * np.eye(K) - alpha * AtA)

    def diag4(Wt):
        out = np.zeros((128, 128), dtype=np.float32)
        for g in range(4):
            out[K * g:K * (g + 1), K * g:K * (g + 1)] = Wt.astype(np.float32)
        return out

    w1 = diag4(W1.T)
    wc = diag4(Wc.T)
    wp = (-beta * np.eye(128)).astype(np.float32)
    idm = np.eye(128, dtype=np.float32)

    A16 = A.astype(np.float16)
    apad = np.zeros((128, 16 * 128), dtype=np.float16)
    for g in range(4):
        for c in range(4):
            blk = np.zeros((128, 128), dtype=np.float16)
            blk[:, K * g:K * (g + 1)] = A16[128 * c:128 * (c + 1), :]
            apad[:, 128 * (4 * g + c):128 * (4 * g + c + 1)] = blk
    wts = np.concatenate([idm, w1, wc, wp], axis=1)
    return apad, wts, float(c0), float(alpha)


_PROGRAM_CACHE = {}


def _get_program(ns, nstep, c0, alpha):
    key = (ns, nstep, round(c0, 10), round(alpha, 12), str(MM_DTYPE))
    if key not in _PROGRAM_CACHE:
        _PROGRAM_CACHE[key] = build_program(ns, nstep, c0, alpha)
    return _PROGRAM_CACHE[key]


def kernel(X: np.ndarray, A: np.ndarray) -> np.ndarray:
    global LAST_RESULTS
    X = np.ascontiguousarray(np.asarray(X, dtype=np.float32))
    A = np.ascontiguousarray(np.asarray(A, dtype=np.float32))
    assert X.shape == (M, N_FULL) and A.shape == (M, K)

    ns = N_FULL // N_CORES
    apad, wts, c0, alpha = host_prep(A, NSTEP)
    nc = _get_program(ns, NSTEP, c0, alpha)

    in_maps = []
    for c in range(N_CORES):
        in_maps.append({
            "x": np.ascontiguousarray(X[:, c * ns:(c + 1) * ns].astype(np.float16)),
            "apad": apad,
            "wts": wts,
        })

    res = run_bass_kernel_spmd(nc, in_maps, core_ids=list(range(N_CORES)))
    LAST_RESULTS = res
    S = np.concatenate([res.results[c]["s_out"] for c in range(N_CORES)], axis=1)
    return np.ascontiguousarray(S.astype(np.float32))


# revision 20
# speedup vs baseline: 1.0457x; 1.0231x over previous
"""Trainium2 Bass kernel for nn_LsqNonneg: batched NNLS via heavy-ball projected gradient.

Math: the reference runs 200 FISTA iterations converging to the NNLS solution
S* (within ~3e-3 of it).  We converge to the same fixed point with a warm
start + constant-momentum heavy-ball iteration, which needs only ~29 steps:

    AtA = A.T A,  eigs: L = lam_max, mu = lam_min
    alpha = 4/(sqrt(L)+sqrt(mu))^2,  beta = ((sqrt(k)-1)/(sqrt(k)+1))^2, k=L/mu
    B   = alpha * A.T X                    [32, N]
    S0  = relu((1/L) A.T X) = relu(c0*B),  c0 = 1/(L*alpha)
    S1  = relu(W S0 + B),                  W  = I - alpha*AtA
    S_{k+1} = relu(Wc S_k - beta*S_{k-1} + B),  Wc = (1+beta)I - alpha*AtA

All iteration weights are constant -> loaded once, no per-iteration streaming.

Device layout (per core, NS=4096 columns): packed [128, 512] per slice s:
partition group g (rows 32g..32g+31) of slice s holds original columns
[g*1024 + 512*s, g*1024 + 512*s + 512).  Weights are diag4 [128,128] blocks so
one full-array matmul advances 4 column blocks; one slice = one PSUM bank.
Per step per slice: 3 accumulating matmuls in order (ident@B start, wp@S_{k-1},
wc@S_k stop -- the relu-gated matmul goes last so the two ungated ones absorb
the relu latency), then relu psum->S (slice 0 on VectorE, slice 1 on ScalarE)
overlapped with the other slice's matmuls.  X is staged in fp16 (host-cast) to
halve the DMA-bound prologue; B accumulates in fp32 PSUM so the data term
keeps full precision.
"""

import os
import sys

import numpy as np

for _p in ("/opt/trn_rl_repo", "/root/.axon_site/_ro/trn_rl_repo"):
    if os.path.isdir(_p) and _p not in sys.path:
        sys.path.append(_p)

from contextlib import ExitStack

import concourse.bass as bass
import concourse.bacc as bacc
import concourse.tile as tile
from concourse import mybir
from concourse.bass_utils import run_bass_kernel_spmd

M, K, N_FULL, N_CORES = 512, 32, 32768, 8
NSTEP = 27               # heavy-ball steps after the warm start
N1 = 4                   # phase-1 steps (overrelaxed alpha), rest at optimal alpha
A1S = 1.8                # phase-1 alpha multiplier

F32 = mybir.dt.float32
F32R = mybir.dt.float32r
F16 = mybir.dt.float16

MM_DTYPE = F32R

LAST_RESULTS = None  # BassKernelResults of the most recent run (for test.py)


def build_program(ns: int, nstep: int, c0: float, alpha: float, mm_dtype=MM_DTYPE):
    """Build the SPMD Bass program for one core holding `ns` columns."""
    DT = mm_dtype
    assert ns == 4096
    SL = 512             # columns per slice (one PSUM bank)
    NSL = 2              # slices

    nc = bacc.Bacc("TRN2", target_bir_lowering=False)

    x_d = nc.dram_tensor("x", [M, ns], F16, kind="ExternalInput")
    apad_d = nc.dram_tensor("apad", [128, 16 * 128], F16, kind="ExternalInput")
    wts_d = nc.dram_tensor("wts", [128, 5 * 128], F32, kind="ExternalInput")
    out_d = nc.dram_tensor("s_out", [K, ns], F32, kind="ExternalOutput")

    with ExitStack() as ctx:
        tc = ctx.enter_context(tile.TileContext(nc))
        persist = ctx.enter_context(tc.tile_pool(name="persist", bufs=1))
        psum = ctx.enter_context(tc.tile_pool(name="psum", bufs=4, space="PSUM"))

        # --- X staging first (critical path), natural layout, fp16 ---
        xts = [persist.tile([128, ns], F16, name=f"xt{c}") for c in range(4)]
        for c in range(4):
            nc.sync.dma_start(xts[c][:], x_d[128 * c:128 * (c + 1), :])

        # --- constants on the scalar HWDGE ring (parallel with X) ---
        apc = persist.tile([128, 16 * 128], F16)  # (g,c) chunk at 128*(4g+c)
        nc.scalar.dma_start(apc[:], apad_d[:])
        wts_sb = persist.tile([128, 5 * 128], DT)
        nc.scalar.dma_start(wts_sb[:], wts_d[:].bitcast(DT))
        id_sb = wts_sb[:, 0:128]       # identity (phase-2 B inject)
        idr_sb = wts_sb[:, 128:256]    # (alpha1/alpha0) * identity (phase 1)
        wc1_sb = wts_sb[:, 256:384]    # (1+b)I - alpha1*AtA, diag4
        wc2_sb = wts_sb[:, 384:512]    # (1+b)I - alpha0*AtA, diag4
        wp_sb = wts_sb[:, 512:640]     # -beta * identity

        # --- B = alpha * A.T X  (packed layout), per slice ---
        pb = [psum.tile([128, SL], F32, tag=f"pt{s}", name=f"pb{s}", bufs=4) for s in range(NSL)]
        for c in range(4):
            for s in range(NSL):
                for g in range(4):
                    nc.tensor.matmul(
                        pb[s][:],
                        apc[:, 128 * (4 * g + c):128 * (4 * g + c + 1)],
                        xts[c][:, 1024 * g + SL * s: 1024 * g + SL * (s + 1)],
                        start=(c == 0 and g == 0),
                        stop=(c == 3 and g == 3),
                    )

        b_sb = [persist.tile([128, SL], DT, name=f"b_sb{s}") for s in range(NSL)]
        s_a = [persist.tile([128, SL], DT, name=f"s_a{s}") for s in range(NSL)]
        s_b = [persist.tile([128, SL], DT, name=f"s_b{s}") for s in range(NSL)]

        # drain B = alpha * (A.T X psum) to SBUF, warm-start S0 = relu(c0 * B)
        # slice 1's warm reads its PSUM directly on ScalarE so neither warm
        # blocks the VectorE relu stream on a cross-engine dependency.
        nc.vector.tensor_scalar_mul(b_sb[0][:], pb[0][:], alpha)
        nc.scalar.activation(b_sb[1][:], pb[1][:],
                             mybir.ActivationFunctionType.Copy, scale=alpha)
        nc.vector.tensor_scalar(s_a[0][:], b_sb[0][:], c0, 0.0,
                                mybir.AluOpType.mult, mybir.AluOpType.max)
        nc.scalar.activation(s_a[1][:], pb[1][:],
                             mybir.ActivationFunctionType.Relu,
                             scale=c0 * alpha)

        # --- heavy-ball loop: step k computes S_{k+1} (S_k in cur) ---
        # k=1 is uniform: prev=cur makes wp@cur+wc@cur = (I - alpha*AtA)@cur.
        for k in range(1, nstep + 1):
            cur = s_a if k % 2 == 1 else s_b
            dest = s_b if k % 2 == 1 else s_a
            inj = idr_sb if k <= N1 else id_sb
            wcw = wc1_sb if k <= N1 else wc2_sb
            pts = []
            for s in range(NSL):
                pt = psum.tile([128, SL], F32, tag=f"pt{s}", name=f"pt{s}", bufs=4)
                prev_t = cur[s] if k == 1 else dest[s]
                nc.tensor.matmul(pt[:], inj, b_sb[s][:],
                                 start=True, stop=False)
                nc.tensor.matmul(pt[:], wp_sb, prev_t[:],
                                 start=False, stop=False)
                nc.tensor.matmul(pt[:], wcw, cur[s][:],
                                 start=False, stop=True)
                pts.append(pt)
            nc.vector.tensor_scalar_max(dest[0][:], pts[0][:], 0.0)
            nc.scalar.activation(dest[1][:], pts[1][:],
                                 mybir.ActivationFunctionType.Relu)

        final = s_a if nstep % 2 == 0 else s_b
        for s in range(NSL):
            for g in range(4):
                eng = nc.sync if g % 2 == 0 else nc.scalar
                eng.dma_start(
                    out_d[:, g * 1024 + SL * s: g * 1024 + SL * (s + 1)],
                    final[s][K * g:K * (g + 1), :].bitcast(F32),
                )

    nc.finalize()
    return nc


def host_prep(A: np.ndarray, nstep: int):
    """Heavy-ball coefficients + constant device weights from A."""
    A = np.asarray(A, dtype=np.float32)
    AtA = (A.T @ A).astype(np.float64)
    ev = np.linalg.eigvalsh(AtA)
    L, mu = float(ev[-1]), float(ev[0])
    kap = L / mu
    alpha = 4.0 / (np.sqrt(L) + np.sqrt(mu)) ** 2
    beta = ((np.sqrt(kap) - 1.0) / (np.sqrt(kap) + 1.0)) ** 2
    c0 = (1.0 / L) / alpha

    alpha1 = A1S * alpha
    Wc1 = ((1.0 + beta) * np.eye(K) - alpha1 * AtA)
    Wc2 = ((1.0 + beta) * np.eye(K) - alpha * AtA)

    def diag4(Wt):
        out = np.zeros((128, 128), dtype=np.float32)
        for g in range(4):
            out[K * g:K * (g + 1), K * g:K * (g + 1)] = Wt.astype(np.float32)
        return out

    wc1 = diag4(Wc1.T)
    wc2 = diag4(Wc2.T)
    wp = (-beta * np.eye(128)).astype(np.float32)
    idm = np.eye(128, dtype=np.float32)
    idr = (A1S * np.eye(128)).astype(np.float32)

    A16 = A.astype(np.float16)
    apad = np.zeros((128, 16 * 128), dtype=np.float16)
    for g in range(4):
        for c in range(4):
            blk = np.zeros((128, 128), dtype=np.float16)
            blk[:, K * g:K * (g + 1)] = A16[128 * c:128 * (c + 1), :]
            apad[:, 128 * (4 * g + c):128 * (4 * g + c + 1)] = blk
    wts = np.concatenate([idm, idr, wc1, wc2, wp], axis=1)
    return apad, wts, float(c0), float(alpha)


_PROGRAM_CACHE = {}


def _get_program(ns, nstep, c0, alpha):
    key = (ns, nstep, round(c0, 10), round(alpha, 12), str(MM_DTYPE))
    if key not in _PROGRAM_CACHE:
        _PROGRAM_CACHE[key] = build_program(ns, nstep, c0, alpha)
    return _PROGRAM_CACHE[key]


def kernel(X: np.ndarray, A: np.ndarray) -> np.ndarray:
    global LAST_RESULTS
    X = np.ascontiguousarray(np.asarray(X, dtype=np.float32))
    A = np.ascontiguousarray(np.asarray(A, dtype=np.float32))
    assert X.shape == (M, N_FULL) and A.shape == (M, K)

    ns = N_FULL // N_CORES
    apad, wts, c0, alpha = host_prep(A, NSTEP)
    nc = _get_program(ns, NSTEP, c0, alpha)

    in_maps = []
    for c in range(N_CORES):
        in_maps.append({
            "x": np.ascontiguousarray(X[:, c * ns:(c + 1) * ns].astype(np.float16)),
            "apad": apad,
            "wts": wts,
        })

    res = run_bass_kernel_spmd(nc, in_maps, core_ids=list(range(N_CORES)))
    LAST_RESULTS = res
    S = np.concatenate([res.results[c]["s_out"] for c in range(N_CORES)], axis=1)
    return np.ascontiguousarray(S.astype(np.float32))
